# revision 13
# baseline (speedup 1.0000x reference)
"""Trainium2 Bass kernel for DecoupledGIN (2-layer GIN+GCN message passing + pooling).

Strategy (8 NeuronCores, SPMD):
  - Nodes split across cores by contiguous id range (uniform-random graph -> balanced).
  - Edges assigned to the core owning the TARGET node; per-core aggregation.
  - Aggregation: edges sorted (window, tile, target); per-edge source rows pulled with
    dma_gather (int16 indices, 4 address windows via base offsets) into SBUF; per-128-edge
    one-hot matmuls (fp16 x fp16 -> f32 PSUM) scatter-reduce into per-tile accumulators.
  - Linear layers are applied POST-aggregation (sums commute with linear maps; biases fold
    via degree); GCN's edge_norm folds into a table column s*dinv carried with each row.
  - Layer-0 table is built on host from the (replicated) inputs -> no communication.
    Only one AllGather (fp16 layer-1 table, 4 chunks) + one small AllReduce (pooling).
  - log_softmax head replicated on all cores.
"""

import math

import numpy as np

LAST_WALL_S = None

CFG = dict(N=100000, E=1600000, IN=128, SE=16, H=64, OUT=16, G=512)
NC = 8
ROWF = 192          # useful row floats: [x|s pack | s*dinv | dinv | pad]
ROWS = 256          # table row stride (f16 -> 512B, dma_gather needs 256B multiples)
NI = 2048           # indices per dma_gather call
BLK = 128           # edges per one-hot matmul block
BPC = NI // BLK     # blocks per call


def _derive(cfg):
    N = cfg["N"]
    per_core = math.ceil(N / NC)
    ntile = math.ceil(per_core / BLK)
    nsh = ntile * BLK
    npad = NC * nsh
    span = math.ceil(npad / 4)
    assert span <= 32767
    return per_core, ntile, nsh, npad, span


def _preprocess(x, s, edge_index, batch, dinv, cfg):
    """Build per-core gather streams + static block structure (shared across cores)."""
    N, E, IN, SE = cfg["N"], cfg["E"], cfg["IN"], cfg["SE"]
    per_core, ntile, nsh, npad, span = _derive(cfg)
    row = np.asarray(edge_index[0], dtype=np.int64)
    col = np.asarray(edge_index[1], dtype=np.int64)

    # old node id -> padded new id
    core_of = np.minimum(np.arange(N) // per_core, NC - 1)
    local_of = np.arange(N) - core_of * per_core
    new_id = core_of * nsh + local_of

    nrow = new_id[row]
    ncol = new_id[col]
    ecore = (ncol // nsh).astype(np.int64)
    etile = ((ncol % nsh) // BLK).astype(np.int64)
    etgt = (ncol % BLK).astype(np.int64)
    ewin = (nrow // span).astype(np.int64)

    # per (core,tile,win) counts -> shared block structure
    key = (ecore * ntile + etile) * 4 + ewin
    cnts = np.bincount(key, minlength=NC * ntile * 4).reshape(NC, ntile, 4)
    btw = np.maximum(1, np.ceil(cnts.max(axis=0) / BLK).astype(np.int64))  # [ntile, 4]

    # window-major stream layout: for w: for t: btw[t,w] blocks
    # block order & call mapping (shared by all cores)
    blocks = []          # (w, t, start, stop)
    call_base = []       # per call: window base (node ids)
    call_of_block = []
    col_of_block = []
    nb_in_call = 0
    ncalls = 0
    for w in range(4):
        first_block_of_call = True
        for t in range(ntile):
            B = int(btw[t, w])
            for b in range(B):
                if nb_in_call == 0:
                    ncalls += 1
                    call_base.append(w * span)
                call_of_block.append(ncalls - 1)
                col_of_block.append(nb_in_call)
                blocks.append((w, t, b == 0, b == B - 1))
                nb_in_call = (nb_in_call + 1) % BPC
        nb_in_call = 0  # calls do not cross windows
    nblocks = len(blocks)

    # per-core streams
    order = np.lexsort((etgt, etile, ewin, ecore))
    ro, to, wo, go, co = nrow[order], etile[order], ewin[order], etgt[order], ecore[order]
    idx16 = np.zeros((NC, ncalls, 128, NI // 16), dtype=np.int16)
    tgtid = np.full((NC, ncalls, 128, BPC), 255.0, dtype=np.float16)

    # slot offsets: for each (c,t,w) group, its edges land in the blocks of (w,t)
    # compute flat slot position within the (w,t) region, then map to (call,col,lane)
    # region start (in blocks) for (w,t):
    reg_start = np.zeros((4, ntile), dtype=np.int64)
    k = 0
    for w in range(4):
        for t in range(ntile):
            reg_start[w, t] = k
            k += int(btw[t, w])
    # global edge slot index within its core's stream:
    grp = (co * 4 + wo) * ntile + to
    # rank of edge within its (c,w,t) group (order is sorted by (c,w,t))
    gcount = np.bincount(grp, minlength=NC * 4 * ntile)
    gstart = np.concatenate([[0], np.cumsum(gcount)[:-1]])
    rank = np.arange(len(order)) - gstart[grp]
    blk_of_edge = reg_start[wo, to] + rank // BLK
    lane_of_edge = rank % BLK
    call_arr = np.asarray(call_of_block, dtype=np.int64)
    colarr = np.asarray(col_of_block, dtype=np.int64)
    ecall = call_arr[blk_of_edge]
    ecol = colarr[blk_of_edge]
    local_idx = (ro - np.asarray(call_base, dtype=np.int64)[ecall]).astype(np.int16)
    # idx16 wrapped layout: stream pos i (= ecol*128+lane) -> [i%16, i//16], replicated x8
    pos = ecol * BLK + lane_of_edge
    p16 = (pos % 16).astype(np.int64)
    s16 = (pos // 16).astype(np.int64)
    for r in range(8):
        idx16[co, ecall, 16 * r + p16, s16] = local_idx
    tgtid[co, ecall, lane_of_edge, ecol] = go.astype(np.float16)

    # layer-0 table (replicated, fp16): [x | s | s*dinv | dinv | 0]
    t0 = np.zeros((npad, ROWS), dtype=np.float16)
    xs = np.asarray(x, dtype=np.float32)
    ss = np.asarray(s, dtype=np.float32)
    t0[new_id, 0:IN] = xs.astype(np.float16)
    t0[new_id, IN : IN + SE] = ss.astype(np.float16)
    t0[new_id, IN + SE : IN + 2 * SE] = (ss * dinv[:, None]).astype(np.float16)
    t0[new_id, IN + 2 * SE] = dinv.astype(np.float16)

    # per-core node tables
    deg_gcn = np.zeros((NC, nsh), dtype=np.float32)
    dinv_t = np.ones((NC, nsh), dtype=np.float32)
    bid = np.full((NC, nsh), 60000.0, dtype=np.float32)
    indeg = np.bincount(col, minlength=N).astype(np.float32)
    deg_gcn_all = indeg + 1.0
    deg_gcn[core_of, local_of] = deg_gcn_all
    deg_gcn[deg_gcn == 0] = 1.0
    dinv_t[core_of, local_of] = dinv
    bid[core_of, local_of] = np.asarray(batch, dtype=np.float32)

    def pack_pt(a):  # [NC, nsh] -> [NC, 128, ntile]
        return np.ascontiguousarray(a.reshape(NC, ntile, BLK).transpose(0, 2, 1))

    meta = dict(
        ncalls=ncalls, nblocks=nblocks, blocks=blocks,
        call_of_block=call_of_block, col_of_block=col_of_block,
        call_base=call_base, ntile=ntile, nsh=nsh, npad=npad, span=span,
    )
    percore = dict(
        idx16=idx16, tgtid=tgtid,
        degf=pack_pt(deg_gcn), dinvf=pack_pt(dinv_t),
        bidf=pack_pt(bid).astype(np.float16),
    )
    return t0, percore, meta, new_id


def _build(cfg, meta, weights):
    """Construct the Bacc program. weights: dict of numpy f32 arrays (host-stacked)."""
    from contextlib import ExitStack

    import concourse.bass as bass
    import concourse.mybir as mybir
    import concourse.tile as tile
    from concourse.tile import add_dep_helper
    from concourse import bacc

    dt = mybir.dt
    f32, f32r, f16, i16 = dt.float32, dt.float32r, dt.float16, dt.int16
    IN, SE, H, OUT, G = cfg["IN"], cfg["SE"], cfg["H"], cfg["OUT"], cfg["G"]
    ntile, nsh, npad, span = meta["ntile"], meta["nsh"], meta["npad"], meta["span"]
    ncalls, blocks = meta["ncalls"], meta["blocks"]
    call_of_block, col_of_block = meta["call_of_block"], meta["col_of_block"]
    call_base = meta["call_base"]
    GC = G // 128 if G >= 128 else 1   # graph chunks
    assert G % 128 == 0 or G < 128
    XS = IN + SE        # 144: x|s width in table rows

    nc = bacc.Bacc(trn_type="TRN2", target_bir_lowering=False, debug=False,
                   num_devices=NC, num_swdge_queues=4)

    t0_t = nc.dram_tensor("t0", [npad, ROWS], f16, kind="ExternalInput")
    self0_t = nc.dram_tensor("self0", [nsh, ROWS], f16, kind="ExternalInput")
    idx_t = nc.dram_tensor("idx16", [ncalls, 128, NI // 16], i16, kind="ExternalInput")
    tgt_t = nc.dram_tensor("tgtid", [ncalls, 128, BPC], f16, kind="ExternalInput")
    degf_t = nc.dram_tensor("degf", [128, ntile], f32, kind="ExternalInput")
    dinvf_t = nc.dram_tensor("dinvf", [128, ntile], f32, kind="ExternalInput")
    bidf_t = nc.dram_tensor("bidf", [128, ntile], f16, kind="ExternalInput")
    iota512_t = nc.dram_tensor("iota512", [128, max(G, 128)], f16, kind="ExternalInput")
    iota128_t = nc.dram_tensor("iota128", [128, 128], f16, kind="ExternalInput")
    eye_t = nc.dram_tensor("eye", [128, 128], f32r, kind="ExternalInput")
    cnt_t = nc.dram_tensor("cntg", [1, max(G, 128)], f32r, kind="ExternalInput")

    wnames = ["pre_w", "embB", "gcnB", "preB", "g1_0", "g2_0", "gw0",
              "b1_0", "b2_0", "gb0", "g1_1", "g2_1", "gw1", "b1_1", "b2_1", "gb1",
              "whp_w", "whp_bR", "post_w", "post_bC", "ro_w", "ro_bC"]
    BIAS_NMS = {"b1_0", "b2_0", "gb0", "b1_1", "b2_1", "gb1", "post_bC", "ro_bC"}
    w_t = {nm: nc.dram_tensor(nm, list(weights[nm].shape),
                              f32 if nm in BIAS_NMS else f32r, kind="ExternalInput")
           for nm in wnames}

    xp_t = nc.dram_tensor("xp", [max(G, 128), H], f32, kind="ExternalOutput")
    y_t = nc.dram_tensor("y", [max(G, 128), OUT], f32, kind="ExternalOutput")

    AF = mybir.ActivationFunctionType
    OP = mybir.AluOpType

    with tile.TileContext(nc) as tc:
        with ExitStack() as ctx:
            const = ctx.enter_context(tc.tile_pool(name="const", bufs=1))
            dram = ctx.enter_context(tc.tile_pool(name="dram", bufs=1, space="DRAM"))
            idxp = ctx.enter_context(tc.tile_pool(name="idxp", bufs=4))
            slotp = ctx.enter_context(tc.tile_pool(name="slotp", bufs=6))
            ohp = ctx.enter_context(tc.tile_pool(name="ohp", bufs=4))
            accp = ctx.enter_context(tc.tile_pool(name="accp", bufs=ntile))
            mlp = ctx.enter_context(tc.tile_pool(name="mlp", bufs=2))
            psb = ctx.enter_context(tc.tile_pool(name="psb", bufs=2, space="PSUM"))
            ps1 = ctx.enter_context(tc.tile_pool(name="ps1", bufs=1, space="PSUM"))

            # ---------------- constants ----------------
            ws = {}
            for nm in wnames:
                shp = list(weights[nm].shape)
                ws[nm] = const.tile(shp, f32 if nm in BIAS_NMS else f32r,
                                    name=f"w_{nm}", tag=f"w_{nm}")
                nc.sync.dma_start(out=ws[nm][:], in_=w_t[nm][:, :])
            eye = const.tile([128, 128], f32r)
            nc.sync.dma_start(out=eye[:], in_=eye_t[:, :])
            iota512 = const.tile([128, max(G, 128)], f16)
            nc.sync.dma_start(out=iota512[:], in_=iota512_t[:, :])
            iota128 = const.tile([128, 128], f16)
            nc.sync.dma_start(out=iota128[:], in_=iota128_t[:, :])
            cntg = const.tile([1, max(G, 128)], f32r)
            nc.sync.dma_start(out=cntg[:], in_=cnt_t[:, :])
            degs = const.tile([128, ntile], f32)
            nc.sync.dma_start(out=degs[:], in_=degf_t[:, :])
            dinvs = const.tile([128, ntile], f32)
            nc.sync.dma_start(out=dinvs[:], in_=dinvf_t[:, :])
            bids = const.tile([128, ntile], f16)
            nc.sync.dma_start(out=bids[:], in_=bidf_t[:, :])

            shard = dram.tile([nsh, ROWS], f16)
            table1 = dram.tile([npad, ROWS], f16, addr_space="Shared")
            xp_bounce = dram.tile([max(G, 128), H], f32)
            xp_red = dram.tile([max(G, 128), H], f32, addr_space="Shared")

            xp_acc = const.tile([128, GC, H], f32)
            nc.vector.memset(xp_acc[:], 0.0)

            gather_chain = [None, 0]   # [prev_inst, global_index]

            def layer(li, table_dram):
                accs = [None] * ntile
                cur_pblk = [None]
                for w in range(4):
                    cur_call = -1
                    st = it = ids = oh = None
                    for bi, (bw, bt, bstart, bstop) in enumerate(blocks):
                        if bw != w:
                            continue
                        ci, cj = call_of_block[bi], col_of_block[bi]
                        if ci != cur_call:
                            cur_call = ci
                            it = idxp.tile([128, NI // 16], i16, name="it", tag="it")
                            nc.sync.dma_start(out=it[:], in_=idx_t[ci])
                            st = slotp.tile([128, BPC, ROWS], f16, name="st", tag="st")
                            g = nc.gpsimd.dma_gather(
                                st[:], table_dram[call_base[ci]:, :], it[:],
                                NI, NI, ROWS, single_packet=False,
                                queue_num=gather_chain[1] % 4)
                            if gather_chain[0] is not None:
                                add_dep_helper(g.ins, gather_chain[0].ins, sync=False,
                                               reason="swdge lane/queue pairing")
                            gather_chain[0] = g
                            gather_chain[1] += 1
                            ids = idxp.tile([128, BPC], f16, name="ids", tag="ids")
                            nc.sync.dma_start(out=ids[:], in_=tgt_t[ci])
                            oh = ohp.tile([128, BPC, 128], f16, name="oh", tag="oh")
                            nc.vector.tensor_tensor(
                                out=oh[:],
                                in0=ids[:, :, None].to_broadcast([128, BPC, 128]),
                                in1=iota128[:, None, :].to_broadcast([128, BPC, 128]),
                                op=OP.is_equal)
                        if bstart:
                            cur_pblk[0] = psb.tile([128, ROWF], f32, name="pblk", tag="pblk")
                        pblk = cur_pblk[0]
                        nc.tensor.matmul(out=pblk[:], lhsT=oh[:, cj, :], rhs=st[:, cj, 0:ROWF],
                                         start=bstart, stop=bstop)
                        if bstop:
                            if w == 0:
                                a = accp.tile([128, ROWF], f32, name=f"acc{bt}", tag="acc")
                                accs[bt] = a
                                nc.vector.tensor_copy(out=a[:], in_=pblk[:])
                            else:
                                a = accs[bt]
                                nc.vector.tensor_add(out=a[:], in0=a[:], in1=pblk[:])
                            if w == 3:
                                tile_mlp(li, bt, a)
                    if li == 0 and w == 3:
                        pass
                # AllGather handled by caller for li==0

            def tile_mlp(li, t, a):
                # R = acc + self  (f32r)
                selfs = mlp.tile([128, ROWS], f16, name="selfs", tag="selfs")
                nc.sync.dma_start(out=selfs[:], in_=self_dram_cur[0][t * BLK:(t + 1) * BLK, :])
                Rt = mlp.tile([128, ROWF], f32r, name="Rt", tag="Rt")
                nc.vector.tensor_add(out=Rt[:], in0=a[:], in1=selfs[:, 0:ROWF])
                ps_tr = ps1.tile([128, 128], f32r, name="ps_tr", tag="ps_tr")
                if li == 0:
                    # GA: [deg | s16]; GB: [sdS16 | diS]
                    smallA = mlp.tile([128, 1 + SE], f32r, name="smallA", tag="smallA")
                    nc.vector.tensor_copy(out=smallA[:, 0:1], in_=degs[:, t:t + 1])
                    nc.vector.tensor_copy(out=smallA[:, 1:1 + SE], in_=Rt[:, IN:XS])
                    smallB = mlp.tile([128, SE + 1], f32r, name="smallB", tag="smallB")
                    nc.vector.tensor_scalar_mul(smallB[:, 0:SE], Rt[:, XS:XS + SE],
                                                dinvs[:, t:t + 1])
                    nc.vector.tensor_scalar_mul(smallB[:, SE:SE + 1],
                                                Rt[:, XS + SE:XS + SE + 1],
                                                dinvs[:, t:t + 1])
                    nc.tensor.transpose(out=ps_tr[0:1 + SE, :], in_=smallA[:], identity=eye[:])
                    GA = mlp.tile([1 + SE, 128], f32r, name="GA", tag="GA")
                    nc.vector.tensor_copy(out=GA[:], in_=ps_tr[0:1 + SE, :])
                    nc.tensor.transpose(out=ps_tr[0:SE + 1, :], in_=smallB[:], identity=eye[:])
                    GB = mlp.tile([SE + 1, 128], f32r, name="GB", tag="GB")
                    nc.vector.tensor_copy(out=GB[:], in_=ps_tr[0:SE + 1, :])
                    nc.tensor.transpose(out=ps_tr[:], in_=Rt[:, 0:IN], identity=eye[:])
                    F1 = mlp.tile([IN, 128], f32r, name="F1", tag="F1")
                    nc.vector.tensor_copy(out=F1[:], in_=ps_tr[0:IN, :])
                    # gin_in
                    pgA = ps1.tile([64, 128], f32, name="pgA", tag="pm")
                    nc.tensor.matmul(out=pgA[:], lhsT=ws["pre_w"][:], rhs=F1[:],
                                     start=True, stop=False)
                    nc.tensor.matmul(out=pgA[:], lhsT=ws["preB"][:], rhs=GA[0:1, :],
                                     start=False, stop=True)
                    pgB = ps1.tile([64, 128], f32, name="pgB", tag="pm2")
                    nc.tensor.matmul(out=pgB[:], lhsT=ws["embB"][:], rhs=GA[:],
                                     start=True, stop=True)
                    Hs = mlp.tile([128, 128], f32r, name="Hs", tag="Hs")
                    nc.vector.tensor_copy(out=Hs[0:64, :], in_=pgA[:])
                    nc.vector.tensor_copy(out=Hs[64:128, :], in_=pgB[:])
                    g1, g2, b1, b2 = ws["g1_0"], ws["g2_0"], ws["b1_0"], ws["b2_0"]
                    gwl, gbl = ws["gw0"], ws["gb0"]
                else:
                    nc.tensor.transpose(out=ps_tr[:], in_=Rt[:, 0:128], identity=eye[:])
                    Hs = mlp.tile([128, 128], f32r, name="Hs", tag="Hs")
                    nc.vector.tensor_copy(out=Hs[:], in_=ps_tr[:])
                    sdS = mlp.tile([128, H], f32r, name="sdS", tag="sdS")
                    nc.vector.tensor_scalar_mul(sdS[:], Rt[:, 128:192], dinvs[:, t:t + 1])
                    nc.tensor.transpose(out=ps_tr[0:H, :], in_=sdS[:], identity=eye[:])
                    GB = mlp.tile([H, 128], f32r, name="GB1", tag="GB1")
                    nc.vector.tensor_copy(out=GB[:], in_=ps_tr[0:H, :])
                    g1, g2, b1, b2 = ws["g1_1"], ws["g2_1"], ws["b1_1"], ws["b2_1"]
                    gwl, gbl = ws["gw1"], ws["gb1"]
                # GIN MLP
                pm = ps1.tile([64, 128], f32, name="pm", tag="pm")
                nc.tensor.matmul(out=pm[:], lhsT=g1[:], rhs=Hs[:], start=True, stop=True)
                H2 = mlp.tile([64, 128], f32r, name="H2", tag="H2")
                nc.scalar.activation(out=H2[:], in_=pm[:], func=AF.Relu, bias=b1[:])
                pm2 = ps1.tile([64, 128], f32, name="pm2", tag="pm2")
                nc.tensor.matmul(out=pm2[:], lhsT=g2[:], rhs=H2[:], start=True, stop=True)
                stacked = mlp.tile([128, 128], f32r, name="stacked", tag="stacked")
                nc.scalar.activation(out=stacked[0:64, :], in_=pm2[:], func=AF.Relu, bias=b2[:])
                # GCN
                if li == 0:
                    psg = ps1.tile([64, 128], f32, name="psg", tag="pm")
                    nc.tensor.matmul(out=psg[:], lhsT=ws["gcnB"][:], rhs=GB[:],
                                     start=True, stop=True)
                    S1 = mlp.tile([64, 128], f32r, name="S1", tag="S1")
                    nc.vector.tensor_copy(out=S1[:], in_=psg[:])
                    psg2 = ps1.tile([64, 128], f32, name="psg2", tag="pm2")
                    nc.tensor.matmul(out=psg2[:], lhsT=gwl[:], rhs=S1[:], start=True, stop=True)
                    nc.scalar.activation(out=stacked[64:128, :], in_=psg2[:],
                                         func=AF.Tanh, bias=gbl[:])
                else:
                    psg2 = ps1.tile([64, 128], f32, name="psg2b", tag="pm2")
                    nc.tensor.matmul(out=psg2[:], lhsT=gwl[:], rhs=GB[:], start=True, stop=True)
                    nc.scalar.activation(out=stacked[64:128, :], in_=psg2[:],
                                         func=AF.Tanh, bias=gbl[:])
                if li == 0:
                    # table1 row write
                    nc.tensor.transpose(out=ps_tr[:], in_=stacked[:], identity=eye[:])
                    xs_nm = mlp.tile([128, 128], f32r, name="xs_nm", tag="xs_nm")
                    nc.vector.tensor_copy(out=xs_nm[:], in_=ps_tr[:])
                    t1row = mlp.tile([128, ROWS], f16, name="t1row", tag="t1row")
                    nc.vector.tensor_copy(out=t1row[:, 0:128], in_=xs_nm[:])
                    nc.vector.tensor_scalar_mul(t1row[:, 128:192], xs_nm[:, 64:128],
                                                dinvs[:, t:t + 1])
                    nc.vector.memset(t1row[:, 192:256], 0.0)
                    nc.sync.dma_start(out=shard[t * BLK:(t + 1) * BLK, :], in_=t1row[:])
                else:
                    # xcat + pooling
                    pxc = ps1.tile([64, 128], f32, name="pxc", tag="pm")
                    nc.tensor.matmul(out=pxc[:], lhsT=ws["whp_w"][:], rhs=stacked[:],
                                     start=True, stop=True)
                    XC = mlp.tile([64, 128], f32r, name="XC", tag="S1")
                    nc.vector.tensor_copy(out=XC[:], in_=pxc[:])
                    nc.tensor.transpose(out=ps_tr[0:128, 0:64], in_=XC[:], identity=eye[0:64, 0:64])
                    xc_nm = mlp.tile([128, H], f16, name="xc_nm", tag="xc_nm")
                    nc.vector.tensor_copy(out=xc_nm[:], in_=ps_tr[0:128, 0:64])
                    oh5 = ohp.tile([128, max(G, 128)], f16, name="oh5", tag="oh5")
                    nc.vector.tensor_tensor(
                        out=oh5[:], in0=bids[:, t:t + 1].to_broadcast([128, max(G, 128)]),
                        in1=iota512[:], op=OP.is_equal)
                    for c in range(GC):
                        pp = psb.tile([128, H], f32, name="pp", tag="pp")
                        nc.tensor.matmul(out=pp[:], lhsT=oh5[:, c * 128:(c + 1) * 128],
                                         rhs=xc_nm[:], start=True, stop=True)
                        nc.vector.tensor_add(out=xp_acc[:, c, :], in0=xp_acc[:, c, :],
                                             in1=pp[:])

            # ---------------- run layers ----------------
            self_dram_cur = [self0_t]
            layer(0, t0_t)
            nc.gpsimd.collective_compute(
                "AllGather", OP.bypass, replica_groups=[list(range(NC))],
                ins=[shard[:, :].opt()], outs=[table1[:, :].opt()])
            self_dram_cur[0] = shard
            layer(1, table1)

            # ---------------- pooling reduce + head ----------------
            nc.sync.dma_start(
                out=xp_bounce[:, :].rearrange("(c p) f -> p c f", c=GC), in_=xp_acc[:])
            nc.gpsimd.collective_compute(
                "AllReduce", OP.add, replica_groups=[list(range(NC))],
                ins=[xp_bounce[:, :].opt()], outs=[xp_red[:, :].opt()])
            for c in range(GC):
                xpr32 = mlp.tile([128, H], f32, name="xpr32", tag="xpr32")
                nc.sync.dma_start(out=xpr32[:], in_=xp_red[c * 128:(c + 1) * 128, :])
                xpr = mlp.tile([128, H], f32r, name="xpr", tag="xpr")
                nc.vector.tensor_copy(out=xpr[:], in_=xpr32[:])
                ph = ps1.tile([128, H], f32, name="ph", tag="pm")
                nc.tensor.matmul(out=ph[:], lhsT=eye[:], rhs=xpr[:], start=True, stop=False)
                nc.tensor.matmul(out=ph[:], lhsT=cntg[:, c * 128:(c + 1) * 128],
                                 rhs=ws["whp_bR"][:], start=False, stop=True)
                hb = mlp.tile([128, H], f32r, name="hb", tag="hb")
                nc.vector.tensor_copy(out=hb[:], in_=ph[:])
                ps_tr2 = ps1.tile([128, 128], f32r, name="ps_tr2", tag="ps_tr")
                nc.tensor.transpose(out=ps_tr2[0:H, 0:128], in_=hb[:], identity=eye[:])
                Fh = mlp.tile([H, 128], f32r, name="Fh", tag="Fh")
                nc.vector.tensor_copy(out=Fh[:], in_=ps_tr2[0:H, 0:128])
                pq = ps1.tile([H, 128], f32, name="pq", tag="pm")
                nc.tensor.matmul(out=pq[:], lhsT=ws["post_w"][:], rhs=Fh[:],
                                 start=True, stop=True)
                XP = mlp.tile([H, 128], f32r, name="XP", tag="H2")
                nc.scalar.activation(out=XP[:], in_=pq[:], func=AF.Relu, bias=ws["post_bC"][:])
                # xp out (graph-major)
                nc.tensor.transpose(out=ps_tr2[0:128, 0:H], in_=XP[:], identity=eye[0:H, 0:H])
                xpo = mlp.tile([128, H], f32, name="xpo", tag="xpo")
                nc.vector.tensor_copy(out=xpo[:], in_=ps_tr2[0:128, 0:H])
                nc.sync.dma_start(out=xp_t[c * 128:(c + 1) * 128, :], in_=xpo[:])
                # logits
                pl = ps1.tile([OUT, 128], f32, name="pl", tag="pm2")
                nc.tensor.matmul(out=pl[:], lhsT=ws["ro_w"][:], rhs=XP[:], start=True, stop=True)
                Lg = mlp.tile([OUT, 128], f32r, name="Lg", tag="Lg")
                nc.vector.tensor_scalar_add(Lg[:], pl[:], ws["ro_bC"][:])
                nc.tensor.transpose(out=ps_tr2[0:128, 0:OUT], in_=Lg[:], identity=eye[0:OUT, 0:OUT])
                z = mlp.tile([128, OUT], f32, name="z", tag="z")
                nc.vector.tensor_copy(out=z[:], in_=ps_tr2[0:128, 0:OUT])
                mx = mlp.tile([128, 1], f32, name="mx", tag="mx")
                nc.vector.reduce_max(mx[:], z[:], axis=mybir.AxisListType.X)
                z2 = mlp.tile([128, OUT], f32, name="z2", tag="z2")
                nc.vector.tensor_scalar_sub(z2[:], z[:], mx[:])
                ez = mlp.tile([128, OUT], f32, name="ez", tag="ez")
                nc.scalar.activation(out=ez[:], in_=z2[:], func=AF.Exp)
                sm = mlp.tile([128, 1], f32, name="sm", tag="sm")
                nc.vector.reduce_sum(sm[:], ez[:], axis=mybir.AxisListType.X)
                lsm = mlp.tile([128, 1], f32, name="lsm", tag="lsm")
                nc.scalar.activation(out=lsm[:], in_=sm[:], func=AF.Ln)
                yv = mlp.tile([128, OUT], f32, name="yv", tag="yv")
                nc.vector.tensor_scalar_sub(yv[:], z2[:], lsm[:])
                nc.sync.dma_start(out=y_t[c * 128:(c + 1) * 128, :], in_=yv[:])

    nc.compile()
    return nc


def _stack_weights(inp, cfg):
    H = cfg["H"]
    f = lambda a: np.ascontiguousarray(np.asarray(a, dtype=np.float32))
    w = {}
    pre_w, pre_b = f(inp["pre_w"]), f(inp["pre_b"])
    emb_w, emb_b = f(inp["emb_w"]), f(inp["emb_b"])
    w["pre_w"] = pre_w
    w["embB"] = np.vstack([emb_b.reshape(1, H), emb_w])     # rhs [deg; s]
    w["gcnB"] = np.vstack([emb_w, emb_b.reshape(1, H)])     # rhs [sdS; diS]
    w["preB"] = pre_b.reshape(1, H)
    for li in range(2):
        w[f"g1_{li}"] = f(inp["gin_w1"][li])
        w[f"g2_{li}"] = f(inp["gin_w2"][li])
        w[f"gw{li}"] = f(inp["gcn_w"][li])
        w[f"b1_{li}"] = f(inp["gin_b1"][li]).reshape(H, 1)
        w[f"b2_{li}"] = f(inp["gin_b2"][li]).reshape(H, 1)
        w[f"gb{li}"] = f(inp["gcn_b"][li]).reshape(H, 1)
    w["g1_1"], w["g2_1"], w["gw1"] = w["g1_1"], w["g2_1"], w["gw1"]
    w["b1_1"], w["b2_1"], w["gb1"] = w["b1_1"], w["b2_1"], w["gb1"]
    w["whp_w"] = f(inp["whp_w"])
    w["whp_bR"] = f(inp["whp_b"]).reshape(1, H)
    w["post_w"] = f(inp["post_w"])
    w["post_bC"] = f(inp["post_b"]).reshape(H, 1)
    w["ro_w"] = f(inp["ro_w"])
    w["ro_bC"] = f(inp["ro_b"]).reshape(cfg["OUT"], 1)
    return w


def _runner(nc, in_maps, n_cores, reps=1):
    import time as _time
    import jax
    import numpy as np
    from jax.sharding import Mesh, PartitionSpec
    from jax.experimental.shard_map import shard_map
    import concourse.mybir as mybir
    from concourse.bass2jax import _bass_exec_p, install_neuronx_cc_hook, partition_id_tensor

    install_neuronx_cc_hook()
    partition_name = nc.partition_id_tensor.name if nc.partition_id_tensor else None
    in_names, out_names, out_avals = [], [], []
    for alloc in nc.m.functions[0].allocations:
        if not isinstance(alloc, mybir.MemoryLocationSet):
            continue
        name = alloc.memorylocations[0].name
        if alloc.kind == "ExternalInput":
            if name != partition_name:
                in_names.append(name)
        elif alloc.kind == "ExternalOutput":
            out_names.append(name)
            out_avals.append(jax.core.ShapedArray(
                tuple(alloc.tensor_shape), mybir.dt.np(alloc.dtype)))
    n_params = len(in_names)
    all_in = list(in_names) + list(out_names)
    if partition_name is not None:
        all_in.append(partition_name)

    def _body(*args):
        operands = list(args)
        if partition_name is not None:
            operands.append(partition_id_tensor())
        return tuple(_bass_exec_p.bind(
            *operands, out_avals=tuple(out_avals), in_names=tuple(all_in),
            out_names=tuple(out_names), lowering_input_output_aliases=(),
            sim_require_finite=False, sim_require_nnan=False, nc=nc))

    devices = jax.devices()[:n_cores]
    mesh = Mesh(np.asarray(devices), ("core",))
    specs = (PartitionSpec("core"),)
    fn = jax.jit(shard_map(_body, mesh=mesh,
                           in_specs=specs * (n_params + len(out_names)),
                           out_specs=specs * len(out_names), check_rep=False),
                 keep_unused=True)
    concat_in = [np.concatenate([np.asarray(in_maps[c][nm]) for c in range(n_cores)], axis=0)
                 for nm in in_names]
    zeros = [np.zeros((n_cores * a.shape[0], *a.shape[1:]), a.dtype) for a in out_avals]
    args = [jax.device_put(a) for a in concat_in] + [jax.device_put(z) for z in zeros]
    outs = fn(*args)
    jax.block_until_ready(outs)
    tmin = None
    if reps > 1:
        ts = []
        for _ in range(reps):
            t0 = _time.time()
            outs = fn(*args)
            jax.block_until_ready(outs)
            ts.append(_time.time() - t0)
        tmin = min(ts)
    global LAST_WALL_S
    LAST_WALL_S = tmin
    results = []
    for c in range(n_cores):
        d = {}
        for i, nm in enumerate(out_names):
            arr = np.asarray(outs[i])
            per = arr.shape[0] // n_cores
            d[nm] = arr[c * per:(c + 1) * per]
        results.append(d)
    return results


def run(inputs, cfg=None, use_sim=False, reps=1):
    cfg = cfg or CFG
    inp = {k: np.asarray(v) for k, v in inputs.items()}
    N, G, H, OUT = cfg["N"], cfg["G"], cfg["H"], cfg["OUT"]
    col = np.asarray(inp["edge_index"][1], dtype=np.int64)
    indeg = np.bincount(col, minlength=N).astype(np.float32)
    dinv = (1.0 / np.sqrt(indeg + 1.0)).astype(np.float32)

    t0, percore, meta, new_id = _preprocess(
        inp["x"], inp["s"], inp["edge_index"], inp["batch"], dinv, cfg)
    w = _stack_weights(inp, cfg)
    nc = _build(cfg, meta, w)

    Gp = max(G, 128)
    iota512 = np.tile(np.arange(Gp, dtype=np.float16), (128, 1))
    iota128 = np.tile(np.arange(128, dtype=np.float16), (128, 1))
    eye = np.eye(128, dtype=np.float32)
    cnt = np.zeros((1, Gp), np.float32)
    cnt[0, :G] = np.bincount(np.asarray(inp["batch"], dtype=np.int64), minlength=G)

    nsh = meta["nsh"]
    in_maps = []
    for c in range(NC):
        m = dict(t0=t0, self0=t0[c * nsh:(c + 1) * nsh],
                 idx16=percore["idx16"][c], tgtid=percore["tgtid"][c],
                 degf=percore["degf"][c], dinvf=percore["dinvf"][c],
                 bidf=percore["bidf"][c],
                 iota512=iota512, iota128=iota128, eye=eye, cntg=cnt)
        for k, v in w.items():
            m[k] = v
        in_maps.append(m)
    if use_sim:
        from concourse.bass_interp import MultiCoreSim
        sim = MultiCoreSim(nc, NC)
        for c in range(NC):
            for k, v in in_maps[c].items():
                sim.cores[c].tensor(k)[:] = v
        sim.simulate()
        xp = np.asarray(sim.cores[0].mem_tensor("xp"))[:G].astype(np.float32)
        y = np.asarray(sim.cores[0].mem_tensor("y"))[:G].astype(np.float32)
        return xp, y
    results = _runner(nc, in_maps, NC, reps=reps)
    xp = results[0]["xp"][:G].astype(np.float32)
    y = results[0]["y"][:G].astype(np.float32)
    return xp, y


def kernel(**inputs):
    return run(inputs, CFG)
